# revision 30
# baseline (speedup 1.0000x reference)
"""Trainium2 Bass kernel for nn_Decoding_43404939493634 (gnn_message_passing).

Reference computation:
    Z_a = node_embedding[actions_idx]            # [B, 64] gather
    s   = state_embedding @ W_4                  # [B, 1]
    Q   = relu(Z_a * s) @ W_5                    # [B, 1]

Algebraic restructuring: for a row with scalar s,
    relu(z * s) @ W5 = s * (relu(z) @ W5)        if s > 0
                     = s * (min(z, 0) @ W5)      if s <= 0
so with per-node values a0 = relu(node)@W5, a1 = min(node,0)@W5 (computed
on device), the per-batch-row work collapses to
    Q[b] = s[b] * ((s[b] > 0) * d[idx[b]] + a1[idx[b]]),   d = a0 - a1

All 64-wide dot products run on the Tensor engine: the host stages
state/node shards TRANSPOSED in a "slot" layout [128, cols] where
column n of chunk c holds two rows' embeddings (partitions 0-63 /
64-127), cast host-side to bf16 (halves HBM read traffic vs streaming
f32; same numerics as the previous inline-cast design).  A bf16 matmul
with a tiny block-diagonal stationary (16 patterns x 4 PE column
groups) lands each chunk's 1024 dot products on its own pair of PSUM
partitions, filling [128, 512] PSUM banks.

Two SPMD launches on 8 cores (measured 57-64 us total vs 92 us for the
previous f32-stream two-launch version; each launch carries ~6 us of
fixed framework preamble and ~3.5 us of end-barrier cost):
  launch A (fused stream, ~41-46 us): streams the REFERENCED node rows
      (~21.6k/core, 2.8 MiB/core bf16) AND the state shard (6.1
      MiB/core bf16), interleaved, via the two HWDGE queues (sync/act)
      alternating.  relu on DVE, PSUM evacuation via ScalarE Copy +
      DVE/GpSimd subs, PE warmed up with small dummy matmuls (the PE
      clock ramps 1.2 -> 2.4 GHz only after ~16 us of activity — the
      launch is PE-ramp-bound, so fusing both streams keeps PE
      saturated and the cold phase absorbs as many of the total matmul
      cycles as possible).  Offloading dot products to DVE/GpSimd
      fold-trees was tried and measured SLOWER (cross-engine dependency
      stalls); so were per-stream DMA queues and finer PSUM evacuation.
      Outputs: s table (state@W4) + per-node (d, a1) tables.
  host: t2 = tbl[actions_idx] gather (data movement only).
  launch B (combine, ~17 us): loads s (f32) + t2 (bf16), computes
      q = s * ((s>0)*d + a1) on DVE in pipelined column halves.

Host work is data movement only (pad/transpose/permute/take/unique/cast);
every arithmetic op runs on device.
"""

import sys

for _p in ("/opt/trn_rl_repo",):
    if _p not in sys.path:
        sys.path.insert(0, _p)

import numpy as np

import concourse.bacc as bacc
import concourse.mybir as mybir
import concourse.tile as tile

F32 = mybir.dt.float32
BF16 = mybir.dt.bfloat16
ALU = mybir.AluOpType
COPY = mybir.ActivationFunctionType.Copy
P = 128

N_NODES = 200000
BATCH = 400000
EMB = 64
NCORES = 8

BATCH_PC = BATCH // NCORES           # 50000 rows/core

FD = 512                             # matmul moving free dim / psum bank cols
CHUNK_ROWS = 2 * FD                  # rows ("slots") covered per matmul

S_CHUNKS = -(-BATCH_PC // CHUNK_ROWS)   # 49
S_SLOTS = S_CHUNKS * CHUNK_ROWS         # 50176
S_COLS = S_CHUNKS * FD                  # 25088
S_GROUPS = -(-S_CHUNKS // 16)           # 4 psum banks for s

DMA_COLS = 4096                      # 1 MiB (bf16) per streaming DMA
HEAD_COLS = 1024


def _nc(num_devices):
    return bacc.Bacc(
        "TRN2", target_bir_lowering=False, debug=False, num_devices=num_devices
    )


def _dma_tiles(total_cols, first=0):
    """Streaming schedule: small first tiles (short latency to the first
    matmul), then 1 MiB tiles.  `first` hoists the tail columns
    [first, total_cols) to the front so the PSUM group they complete is
    evacuated early, off the combine tail."""
    sched = []
    if first:
        sched.append((first, total_cols - first))
        total_cols = first
    c0 = 0
    for w in (FD, HEAD_COLS):
        if c0 < total_cols:
            cw = min(w, total_cols - c0)
            sched.append((c0, cw))
            c0 += cw
    while c0 < total_cols:
        cw = min(DMA_COLS, total_cols - c0)
        sched.append((c0, cw))
        c0 += cw
    return sched


def build_fused(n_chunks, num_devices=NCORES):
    """Launch A: stream nodes + state (bf16), compute
    s = state@W4, a1 = node@W5 - relu(node)@W5, d = 2*relu(node)@W5 - node@W5."""
    n_cols = n_chunks * FD
    n_groups = -(-n_chunks // 16)
    nc = _nc(num_devices)
    ndT = nc.declare_dram_parameter("ndT", [P, n_cols], BF16, isOutput=False)
    stT = nc.declare_dram_parameter("stT", [P, S_COLS], BF16, isOutput=False)
    patw4 = nc.declare_dram_parameter("patw4", [P, FD], BF16, isOutput=False)
    patw5 = nc.declare_dram_parameter("patw5", [P, FD], BF16, isOutput=False)
    d_out = nc.declare_dram_parameter("d_out", [P, FD], F32, isOutput=True)
    a1_out = nc.declare_dram_parameter("a1_out", [P, FD], F32, isOutput=True)
    s_out = nc.declare_dram_parameter("s_out", [P, FD], F32, isOutput=True)

    with tile.TileContext(nc) as tc:
        with (
            tc.tile_pool(name="const", bufs=1) as cpool,
            tc.tile_pool(name="nwork", bufs=5) as npool,
            tc.tile_pool(name="swork", bufs=9) as spool,
            tc.tile_pool(name="psum", bufs=1, space="PSUM") as ppool,
        ):
            p4 = cpool.tile([P, FD], BF16, tag="p4")
            nc.sync.dma_start(out=p4[:], in_=patw4[:])
            p5 = cpool.tile([P, FD], BF16, tag="p5")
            nc.scalar.dma_start(out=p5[:], in_=patw5[:])

            ps_s = [ppool.tile([P, FD], F32, tag=f"ps_s{g}", name=f"ps_s{g}")
                    for g in range(S_GROUPS)]
            ps_a0 = [ppool.tile([P, FD], F32, tag=f"ps_a0{g}", name=f"ps_a0{g}")
                     for g in range(n_groups)]
            ps_s5 = [ppool.tile([P, FD], F32, tag=f"ps_s5{g}", name=f"ps_s5{g}")
                     for g in range(n_groups)]

            # PE warmup: dummy matmuls during the head window so the HAM
            # clock-gate releases (1.2 -> 2.4 GHz) earlier; measured ~3-4us
            # faster than without.  Uses partitions 0-31 of g2's bank (real
            # g2 data lands in partitions 64-95, and g2 starts last).
            warm = cpool.tile([P, FD], BF16, tag="warm")
            nc.vector.memset(warm[:], 0.0)
            for _ in range(7):
                nc.tensor.matmul(
                    ps_s[2][0:32, 0:256], warm[:, 0:32], warm[:, 0:256],
                    start=True, stop=True, skip_group_check=True,
                    tile_position=(0, 0),
                )

            # evacuation staging tiles
            svt = cpool.tile([P, FD], F32, tag="svt")
            a0t = cpool.tile([P, FD], F32, tag="a0t")
            a1t = cpool.tile([P, FD], F32, tag="a1t")
            dt_ = cpool.tile([P, FD], F32, tag="dt")

            # unified stream schedule: state head (incl. hoisted lone g3
            # chunk) first, then alternate node/state tiles
            nsched = [("n",) + t for t in _dma_tiles(n_cols)]
            ssched = [("s",) + t
                      for t in _dma_tiles(S_COLS, first=(S_CHUNKS - 1) * FD)]
            sched = []
            while ssched or nsched:
                if ssched:
                    sched.append(ssched.pop(0))
                if nsched:
                    sched.append(nsched.pop(0))

            s_left = [16, 16, 16, S_CHUNKS - 48]  # chunks remaining per group
            n_left = [min(16, n_chunks - 16 * g) for g in range(n_groups)]

            qi = 0
            for kind, c0, cw in sched:
                eng = nc.sync if (qi % 2 == 0) else nc.scalar
                qi += 1
                src = ndT if kind == "n" else stT
                pool = npool if kind == "n" else spool
                tb = pool.tile([P, cw], BF16, tag=f"{kind}b")
                eng.dma_start(out=tb[:], in_=src[:, c0:c0 + cw])
                if kind == "n":
                    rl = npool.tile([P, cw], BF16, tag="rl")
                    nc.vector.tensor_scalar_max(out=rl[:], in0=tb[:], scalar1=0.0)
                base = c0 // FD
                done = []
                for k in range(cw // FD):
                    c = base + k
                    g, j = divmod(c, 16)
                    if kind == "s":
                        nc.tensor.matmul(
                            ps_s[g][32 * g:32 * g + 32, :],
                            p4[:, 32 * j:32 * j + 32],
                            tb[:, k * FD:(k + 1) * FD],
                            start=(j == 0),
                            stop=(j == 15) or (c == S_CHUNKS - 1),
                            skip_group_check=True,
                            tile_position=(0, 32 * g),
                        )
                        s_left[g] -= 1
                        if s_left[g] == 0:
                            done.append(("s", g))
                    else:
                        st_flags = dict(
                            start=(j == 0),
                            stop=(j == 15) or (c == n_chunks - 1),
                            skip_group_check=True,
                            tile_position=(0, 32 * g),
                        )
                        nc.tensor.matmul(
                            ps_a0[g][32 * g:32 * g + 32, :],
                            p5[:, 32 * j:32 * j + 32],
                            rl[:, k * FD:(k + 1) * FD],
                            **st_flags,
                        )
                        nc.tensor.matmul(
                            ps_s5[g][32 * g:32 * g + 32, :],
                            p5[:, 32 * j:32 * j + 32],
                            tb[:, k * FD:(k + 1) * FD],
                            **st_flags,
                        )
                        n_left[g] -= 1
                        if n_left[g] == 0:
                            done.append(("n", g))
                # evacuate completed PSUM groups (PSUM reads on scalar/DVE —
                # gpsimd cannot touch PSUM; SBUF-only subtract on gpsimd)
                for dk, g in done:
                    sl = slice(32 * g, 32 * g + 32)
                    if dk == "s":
                        nc.scalar.activation(
                            out=svt[sl, :], in_=ps_s[g][sl, :], func=COPY
                        )
                        nc.sync.dma_start(out=s_out[sl, :], in_=svt[sl, :])
                    else:
                        nc.scalar.activation(
                            out=a0t[sl, :], in_=ps_a0[g][sl, :], func=COPY
                        )
                        nc.vector.tensor_tensor(
                            out=a1t[sl, :], in0=ps_s5[g][sl, :], in1=a0t[sl, :],
                            op=ALU.subtract,
                        )
                        nc.gpsimd.tensor_tensor(
                            out=dt_[sl, :], in0=a0t[sl, :], in1=a1t[sl, :],
                            op=ALU.subtract,
                        )
                        nc.scalar.dma_start(out=a1_out[sl, :], in_=a1t[sl, :])
                        nc.scalar.dma_start(out=d_out[sl, :], in_=dt_[sl, :])
    nc.compile()
    return nc


def build_combine(num_devices=NCORES):
    """Launch B: q = s * ((s>0)*d + a1), with (d, a1) host-gathered per row."""
    nc = _nc(num_devices)
    sv = nc.declare_dram_parameter("sv", [P, FD], F32, isOutput=False)
    t2 = nc.declare_dram_parameter("t2", [P, FD, 2], BF16, isOutput=False)
    q = nc.declare_dram_parameter("q", [P, FD], F32, isOutput=True)

    NH = 2                                   # column halves, pipelined
    HW_ = FD // NH
    with tile.TileContext(nc) as tc:
        with tc.tile_pool(name="const", bufs=1) as cpool:
            svt = cpool.tile([P, FD], F32, tag="svt")
            t2t = cpool.tile([P, FD, 2], BF16, tag="t2t")
            posm = cpool.tile([P, FD], F32, tag="posm")
            sel = cpool.tile([P, FD], F32, tag="sel")
            qt = cpool.tile([P, FD], F32, tag="qt")
            for h in range(NH):
                cs = slice(h * HW_, (h + 1) * HW_)
                nc.sync.dma_start(out=t2t[:, cs, :], in_=t2[:, cs, :])
                nc.scalar.dma_start(out=svt[:, cs], in_=sv[:, cs])
            for h in range(NH):
                cs = slice(h * HW_, (h + 1) * HW_)
                nc.vector.scalar_tensor_tensor(
                    out=posm[:, cs], in0=svt[:, cs], scalar=0.0,
                    in1=t2t[:, cs, 0], op0=ALU.is_gt, op1=ALU.mult,
                )
                nc.vector.tensor_tensor(
                    out=sel[:, cs], in0=posm[:, cs], in1=t2t[:, cs, 1],
                    op=ALU.add,
                )
                nc.vector.tensor_tensor(
                    out=qt[:, cs], in0=svt[:, cs], in1=sel[:, cs], op=ALU.mult
                )
                nc.sync.dma_start(out=q[:, cs], in_=qt[:, cs])
    nc.compile()
    return nc


# ---------------------------------------------------------------------------
# host-side staging (data movement only) + execution

_CACHE = {}
LAST_RUNS = []  # BassKernelResults of each launch in the last kernel() call


def _runner(key, build_fn):
    if key not in _CACHE:
        _CACHE[key] = build_fn()
    return _CACHE[key]


def _run_spmd(nc, in_maps):
    from concourse.bass_utils import run_bass_kernel_spmd

    r = run_bass_kernel_spmd(nc, in_maps, core_ids=list(range(NCORES)))
    LAST_RUNS.append(r)
    return r.results


def _slotT(rows, n_slots, nch):
    """[n, 64] -> transposed slot layout [128, n_slots//2] bf16: column of
    chunk c, col n holds rows (1024c+2n) on partitions 0-63 and (1024c+2n+1)
    on 64-127."""
    n = rows.shape[0]
    buf = np.zeros((n_slots, EMB), np.float32)
    buf[:n] = rows
    arr = buf.reshape(nch, FD, 2, EMB)           # [c, n, h, e]
    return np.ascontiguousarray(
        arr.transpose(2, 3, 0, 1).reshape(P, nch * FD)
    ).astype(mybir.dt.np(BF16))


def _pidx(n_chunks):
    """Partition index of (chunk c, half h) in the psum/slot output layout."""
    c = np.arange(n_chunks)[:, None]
    h = np.arange(2)[None, :]
    return (32 * (c // 16) + 2 * (c % 16) + h)   # [n_chunks, 2]


def _unslot(mat, n_chunks):
    """[128, 512] device output -> flat [n_chunks*1024] slot-ordered values."""
    pi = _pidx(n_chunks).reshape(-1)             # [2*n_chunks]
    v = mat[pi, :].reshape(n_chunks, 2, FD)      # [c, h, n]
    return np.ascontiguousarray(v.transpose(0, 2, 1)).reshape(-1)


def _slot_pairs(pairs, n_chunks):
    """[n_slots, 2] per-slot values -> [128, 512, 2] device layout."""
    pi = _pidx(n_chunks).reshape(-1)
    arr = pairs.reshape(n_chunks, FD, 2, 2)      # [c, n, h, v]
    out = np.zeros((P, FD, 2), np.float32)
    out[pi] = arr.transpose(0, 2, 1, 3).reshape(2 * n_chunks, FD, 2)
    return out


def _patterns(w):
    """16 block-diagonal stationaries packed as [128, 512] bf16: pattern j in
    cols [32j, 32j+32) with w at (rows 0-63, col 2j), (rows 64-127, col
    2j+1)."""
    pat = np.zeros((P, FD), np.float32)
    for j in range(16):
        pat[:EMB, 32 * j + 2 * j] = w
        pat[EMB:, 32 * j + 2 * j + 1] = w
    return pat


def kernel(actions_idx, node_embedding, state_embedding, W_4, W_5):
    LAST_RUNS.clear()
    actions_idx = np.asarray(actions_idx)
    node_embedding = np.ascontiguousarray(np.asarray(node_embedding, dtype=np.float32))
    state_embedding = np.ascontiguousarray(np.asarray(state_embedding, dtype=np.float32))
    w4 = np.asarray(W_4, dtype=np.float32).reshape(EMB)
    w5 = np.asarray(W_5, dtype=np.float32).reshape(EMB)
    bf16 = mybir.dt.np(BF16)
    patw4 = _patterns(w4).astype(bf16)
    patw5 = _patterns(w5).astype(bf16)

    # ---- launch A: fused node+state stream (only referenced nodes staged)
    uniq, inv = np.unique(actions_idx, return_inverse=True)
    u_pc = -(-len(uniq) // NCORES)               # referenced nodes per core
    n_chunks = max(1, -(-u_pc // CHUNK_ROWS))    # 22 for the target workload
    n_slots = n_chunks * CHUNK_ROWS
    ncA = _runner(("fused", n_chunks), lambda: build_fused(n_chunks))
    inA = []
    for c in range(NCORES):
        rows = node_embedding[uniq[c * u_pc:(c + 1) * u_pc]]
        inA.append({
            "ndT": _slotT(rows, n_slots, n_chunks),
            "stT": _slotT(state_embedding[c * BATCH_PC:(c + 1) * BATCH_PC],
                          S_SLOTS, S_CHUNKS),
            "patw4": patw4,
            "patw5": patw5,
        })
    resA = _run_spmd(ncA, inA)

    tblu = np.empty((NCORES * u_pc, 2), np.float32)
    for c in range(NCORES):
        sl = slice(c * u_pc, (c + 1) * u_pc)
        tblu[sl, 0] = _unslot(resA[c]["d_out"], n_chunks)[:u_pc]
        tblu[sl, 1] = _unslot(resA[c]["a1_out"], n_chunks)[:u_pc]

    # ---- launch B: combine
    ncB = _runner("combine", build_combine)
    inB = []
    for c in range(NCORES):
        cinv = inv[c * BATCH_PC:(c + 1) * BATCH_PC]
        pairs = np.zeros((S_SLOTS, 2), np.float32)
        pairs[:BATCH_PC] = tblu[cinv]
        inB.append({
            "sv": resA[c]["s_out"],
            "t2": _slot_pairs(pairs, S_CHUNKS).astype(bf16),
        })
    resB = _run_spmd(ncB, inB)

    out = np.empty(BATCH, np.float32)
    for c in range(NCORES):
        out[c * BATCH_PC:(c + 1) * BATCH_PC] = \
            _unslot(resB[c]["q"], S_CHUNKS)[:BATCH_PC]
    return out.reshape(BATCH, 1)


# revision 36
# speedup vs baseline: 1.0160x; 1.0160x over previous
"""Trainium2 Bass kernel for nn_Decoding_43404939493634 (gnn_message_passing).

Reference computation:
    Z_a = node_embedding[actions_idx]            # [B, 64] gather
    s   = state_embedding @ W_4                  # [B, 1]
    Q   = relu(Z_a * s) @ W_5                    # [B, 1]

Algebraic restructuring: for a row with scalar s,
    relu(z * s) @ W5 = s * (relu(z) @ W5)        if s > 0
                     = s * (min(z, 0) @ W5)      if s <= 0
so with per-node values a0 = relu(node)@W5, a1 = min(node,0)@W5 (computed
on device), the per-batch-row work collapses to
    Q[b] = s[b] * ((s[b] > 0) * d[idx[b]] + a1[idx[b]]),   d = a0 - a1

All 64-wide dot products run on the Tensor engine: the host stages
state/node shards TRANSPOSED in a "slot" layout [128, cols] where
column n of chunk c holds two rows' embeddings (partitions 0-63 /
64-127), cast host-side to bf16 (halves HBM read traffic vs streaming
f32; same numerics as the previous inline-cast design).  A bf16 matmul
with a tiny block-diagonal stationary (16 patterns x 4 PE column
groups) lands each chunk's 1024 dot products on its own pair of PSUM
partitions, filling [128, 512] PSUM banks.

Two SPMD launches on 8 cores (measured 57-64 us total vs 92 us for the
previous f32-stream two-launch version; each launch carries ~6 us of
fixed framework preamble and ~3.5 us of end-barrier cost):
  launch A (fused stream, ~41-46 us): streams the REFERENCED node rows
      (~21.6k/core, 2.8 MiB/core bf16) AND the state shard (6.1
      MiB/core bf16), interleaved, via the two HWDGE queues (sync/act)
      alternating.  relu on DVE, PSUM evacuation via ScalarE Copy +
      DVE/GpSimd subs, PE warmed up with small dummy matmuls (the PE
      clock ramps 1.2 -> 2.4 GHz only after ~16 us of activity — the
      launch is PE-ramp-bound, so fusing both streams keeps PE
      saturated and the cold phase absorbs as many of the total matmul
      cycles as possible).  Offloading dot products to DVE/GpSimd
      fold-trees was tried and measured SLOWER (cross-engine dependency
      stalls); so were per-stream DMA queues and finer PSUM evacuation.
      Outputs: s table (state@W4) + per-node (d, a1) tables.
  host: t2 = tbl[actions_idx] gather (data movement only).
  launch B (combine, ~17 us): loads s (f32) + t2 (bf16), computes
      q = s * ((s>0)*d + a1) on DVE in pipelined column halves.

Host work is data movement only (pad/transpose/permute/take/unique/cast);
every arithmetic op runs on device.
"""

import sys

for _p in ("/opt/trn_rl_repo",):
    if _p not in sys.path:
        sys.path.insert(0, _p)

import numpy as np

import concourse.bacc as bacc
import concourse.mybir as mybir
import concourse.tile as tile

F32 = mybir.dt.float32
BF16 = mybir.dt.bfloat16
ALU = mybir.AluOpType
COPY = mybir.ActivationFunctionType.Copy
P = 128

N_NODES = 200000
BATCH = 400000
EMB = 64
NCORES = 8

BATCH_PC = BATCH // NCORES           # 50000 rows/core

FD = 512                             # matmul moving free dim / psum bank cols
CHUNK_ROWS = 2 * FD                  # rows ("slots") covered per matmul

S_CHUNKS = -(-BATCH_PC // CHUNK_ROWS)   # 49
S_SLOTS = S_CHUNKS * CHUNK_ROWS         # 50176
S_COLS = S_CHUNKS * FD                  # 25088
S_GROUPS = -(-S_CHUNKS // 16)           # 4 psum banks for s

DMA_COLS = 4096                      # 1 MiB (bf16) per streaming DMA
HEAD_COLS = 1024


def _nc(num_devices):
    return bacc.Bacc(
        "TRN2", target_bir_lowering=False, debug=False, num_devices=num_devices
    )


def _dma_tiles(total_cols, first=0):
    """Streaming schedule: small first tiles (short latency to the first
    matmul), then 1 MiB tiles.  `first` hoists the tail columns
    [first, total_cols) to the front so the PSUM group they complete is
    evacuated early, off the combine tail."""
    sched = []
    if first:
        sched.append((first, total_cols - first))
        total_cols = first
    c0 = 0
    for w in (FD, HEAD_COLS):
        if c0 < total_cols:
            cw = min(w, total_cols - c0)
            sched.append((c0, cw))
            c0 += cw
    while c0 < total_cols:
        cw = min(DMA_COLS, total_cols - c0)
        sched.append((c0, cw))
        c0 += cw
    return sched


def build_fused(n_chunks, num_devices=NCORES):
    """Launch A: stream nodes + state (bf16), compute
    s = state@W4, a1 = node@W5 - relu(node)@W5, d = 2*relu(node)@W5 - node@W5."""
    n_cols = n_chunks * FD
    n_groups = -(-n_chunks // 16)
    nc = _nc(num_devices)
    ndT = nc.declare_dram_parameter("ndT", [P, n_cols], BF16, isOutput=False)
    stT = nc.declare_dram_parameter("stT", [P, S_COLS], BF16, isOutput=False)
    patw4 = nc.declare_dram_parameter("patw4", [P, FD], BF16, isOutput=False)
    patw5 = nc.declare_dram_parameter("patw5", [P, FD], BF16, isOutput=False)
    d_out = nc.declare_dram_parameter("d_out", [P, FD], F32, isOutput=True)
    a1_out = nc.declare_dram_parameter("a1_out", [P, FD], F32, isOutput=True)
    s_out = nc.declare_dram_parameter("s_out", [P, FD], BF16, isOutput=True)

    with tile.TileContext(nc) as tc:
        with (
            tc.tile_pool(name="const", bufs=1) as cpool,
            tc.tile_pool(name="nwork", bufs=5) as npool,
            tc.tile_pool(name="swork", bufs=9) as spool,
            tc.tile_pool(name="psum", bufs=1, space="PSUM") as ppool,
        ):
            p4 = cpool.tile([P, FD], BF16, tag="p4")
            nc.sync.dma_start(out=p4[:], in_=patw4[:])
            p5 = cpool.tile([P, FD], BF16, tag="p5")
            nc.scalar.dma_start(out=p5[:], in_=patw5[:])

            ps_s = [ppool.tile([P, FD], F32, tag=f"ps_s{g}", name=f"ps_s{g}")
                    for g in range(S_GROUPS)]
            ps_a0 = [ppool.tile([P, FD], F32, tag=f"ps_a0{g}", name=f"ps_a0{g}")
                     for g in range(n_groups)]
            ps_s5 = [ppool.tile([P, FD], F32, tag=f"ps_s5{g}", name=f"ps_s5{g}")
                     for g in range(n_groups)]

            # PE warmup: dummy matmuls during the head window so the HAM
            # clock-gate releases (1.2 -> 2.4 GHz) earlier; measured ~3-4us
            # faster than without.  Uses partitions 0-31 of g2's bank (real
            # g2 data lands in partitions 64-95, and g2 starts last).
            warm = cpool.tile([P, FD], BF16, tag="warm")
            nc.vector.memset(warm[:], 0.0)
            for _ in range(7):
                nc.tensor.matmul(
                    ps_s[2][0:32, 0:256], warm[:, 0:32], warm[:, 0:256],
                    start=True, stop=True, skip_group_check=True,
                    tile_position=(0, 0),
                )

            # evacuation staging tiles
            svt = cpool.tile([P, FD], BF16, tag="svt")
            a0t = cpool.tile([P, FD], F32, tag="a0t")
            a1t = cpool.tile([P, FD], F32, tag="a1t")
            dt_ = cpool.tile([P, FD], F32, tag="dt")

            # unified stream schedule: state head (incl. hoisted lone g3
            # chunk) first, then alternate node/state tiles (measured best;
            # all-nodes-first and all-state-first are both ~2-3us slower)
            nsched = [("n",) + t for t in _dma_tiles(n_cols)]
            ssched = [("s",) + t
                      for t in _dma_tiles(S_COLS, first=(S_CHUNKS - 1) * FD)]
            sched = []
            while ssched or nsched:
                if ssched:
                    sched.append(ssched.pop(0))
                if nsched:
                    sched.append(nsched.pop(0))

            s_left = [16, 16, 16, S_CHUNKS - 48]  # chunks remaining per group
            n_left = [min(16, n_chunks - 16 * g) for g in range(n_groups)]

            qi = 0
            for kind, c0, cw in sched:
                eng = nc.sync if (qi % 2 == 0) else nc.scalar
                qi += 1
                src = ndT if kind == "n" else stT
                pool = npool if kind == "n" else spool
                tb = pool.tile([P, cw], BF16, tag=f"{kind}b")
                eng.dma_start(out=tb[:], in_=src[:, c0:c0 + cw])
                if kind == "n":
                    rl = npool.tile([P, cw], BF16, tag="rl")
                    nc.vector.tensor_scalar_max(out=rl[:], in0=tb[:], scalar1=0.0)
                base = c0 // FD
                done = []
                for k in range(cw // FD):
                    c = base + k
                    g, j = divmod(c, 16)
                    if kind == "s":
                        nc.tensor.matmul(
                            ps_s[g][32 * g:32 * g + 32, :],
                            p4[:, 32 * j:32 * j + 32],
                            tb[:, k * FD:(k + 1) * FD],
                            start=(j == 0),
                            stop=(j == 15) or (c == S_CHUNKS - 1),
                            skip_group_check=True,
                            tile_position=(0, 32 * g),
                        )
                        s_left[g] -= 1
                        if s_left[g] == 0:
                            done.append(("s", g))
                    else:
                        st_flags = dict(
                            start=(j == 0),
                            stop=(j == 15) or (c == n_chunks - 1),
                            skip_group_check=True,
                            tile_position=(0, 32 * g),
                        )
                        # s5 first: it reads the raw tile, no relu dep
                        nc.tensor.matmul(
                            ps_s5[g][32 * g:32 * g + 32, :],
                            p5[:, 32 * j:32 * j + 32],
                            tb[:, k * FD:(k + 1) * FD],
                            **st_flags,
                        )
                        nc.tensor.matmul(
                            ps_a0[g][32 * g:32 * g + 32, :],
                            p5[:, 32 * j:32 * j + 32],
                            rl[:, k * FD:(k + 1) * FD],
                            **st_flags,
                        )
                        n_left[g] -= 1
                        if n_left[g] == 0:
                            done.append(("n", g))
                # evacuate completed PSUM groups (PSUM reads on scalar/DVE —
                # gpsimd cannot touch PSUM; SBUF-only subtract on gpsimd)
                for dk, g in done:
                    sl = slice(32 * g, 32 * g + 32)
                    if dk == "s":
                        nc.scalar.activation(
                            out=svt[sl, :], in_=ps_s[g][sl, :], func=COPY
                        )
                        nc.sync.dma_start(out=s_out[sl, :], in_=svt[sl, :])
                    else:
                        nc.scalar.activation(
                            out=a0t[sl, :], in_=ps_a0[g][sl, :], func=COPY
                        )
                        nc.vector.tensor_tensor(
                            out=a1t[sl, :], in0=ps_s5[g][sl, :], in1=a0t[sl, :],
                            op=ALU.subtract,
                        )
                        nc.gpsimd.tensor_tensor(
                            out=dt_[sl, :], in0=a0t[sl, :], in1=a1t[sl, :],
                            op=ALU.subtract,
                        )
                        nc.scalar.dma_start(out=a1_out[sl, :], in_=a1t[sl, :])
                        nc.scalar.dma_start(out=d_out[sl, :], in_=dt_[sl, :])
    nc.compile()
    return nc


def build_combine(num_devices=NCORES):
    """Launch B: q = s * ((s>0)*d + a1), with (d, a1) host-gathered per row."""
    nc = _nc(num_devices)
    sv = nc.declare_dram_parameter("sv", [P, FD], BF16, isOutput=False)
    t2 = nc.declare_dram_parameter("t2", [P, FD, 2], BF16, isOutput=False)
    q = nc.declare_dram_parameter("q", [P, FD], F32, isOutput=True)

    NH = 2                                   # column halves, pipelined
    HW_ = FD // NH
    with tile.TileContext(nc) as tc:
        with tc.tile_pool(name="const", bufs=1) as cpool:
            svt = cpool.tile([P, FD], BF16, tag="svt")
            t2t = cpool.tile([P, FD, 2], BF16, tag="t2t")
            posm = cpool.tile([P, FD], F32, tag="posm")
            sel = cpool.tile([P, FD], F32, tag="sel")
            qt = cpool.tile([P, FD], F32, tag="qt")
            for h in range(NH):
                cs = slice(h * HW_, (h + 1) * HW_)
                nc.sync.dma_start(out=t2t[:, cs, :], in_=t2[:, cs, :])
                nc.scalar.dma_start(out=svt[:, cs], in_=sv[:, cs])
            for h in range(NH):
                cs = slice(h * HW_, (h + 1) * HW_)
                nc.vector.scalar_tensor_tensor(
                    out=posm[:, cs], in0=svt[:, cs], scalar=0.0,
                    in1=t2t[:, cs, 0], op0=ALU.is_gt, op1=ALU.mult,
                )
                nc.vector.tensor_tensor(
                    out=sel[:, cs], in0=posm[:, cs], in1=t2t[:, cs, 1],
                    op=ALU.add,
                )
                nc.vector.tensor_tensor(
                    out=qt[:, cs], in0=svt[:, cs], in1=sel[:, cs], op=ALU.mult
                )
                nc.sync.dma_start(out=q[:, cs], in_=qt[:, cs])
    nc.compile()
    return nc


# ---------------------------------------------------------------------------
# host-side staging (data movement only) + execution

_CACHE = {}
LAST_RUNS = []  # BassKernelResults of each launch in the last kernel() call


def _runner(key, build_fn):
    if key not in _CACHE:
        _CACHE[key] = build_fn()
    return _CACHE[key]


def _run_spmd(nc, in_maps):
    from concourse.bass_utils import run_bass_kernel_spmd

    r = run_bass_kernel_spmd(nc, in_maps, core_ids=list(range(NCORES)))
    LAST_RUNS.append(r)
    return r.results


def _slotT(rows, n_slots, nch):
    """[n, 64] -> transposed slot layout [128, n_slots//2] bf16: column of
    chunk c, col n holds rows (1024c+2n) on partitions 0-63 and (1024c+2n+1)
    on 64-127."""
    n = rows.shape[0]
    buf = np.zeros((n_slots, EMB), np.float32)
    buf[:n] = rows
    arr = buf.reshape(nch, FD, 2, EMB)           # [c, n, h, e]
    return np.ascontiguousarray(
        arr.transpose(2, 3, 0, 1).reshape(P, nch * FD)
    ).astype(mybir.dt.np(BF16))


def _pidx(n_chunks):
    """Partition index of (chunk c, half h) in the psum/slot output layout."""
    c = np.arange(n_chunks)[:, None]
    h = np.arange(2)[None, :]
    return (32 * (c // 16) + 2 * (c % 16) + h)   # [n_chunks, 2]


def _unslot(mat, n_chunks):
    """[128, 512] device output -> flat [n_chunks*1024] slot-ordered values."""
    pi = _pidx(n_chunks).reshape(-1)             # [2*n_chunks]
    v = mat[pi, :].reshape(n_chunks, 2, FD)      # [c, h, n]
    return np.ascontiguousarray(v.transpose(0, 2, 1)).reshape(-1)


def _slot_pairs(pairs, n_chunks):
    """[n_slots, 2] per-slot values -> [128, 512, 2] device layout."""
    pi = _pidx(n_chunks).reshape(-1)
    arr = pairs.reshape(n_chunks, FD, 2, 2)      # [c, n, h, v]
    out = np.zeros((P, FD, 2), np.float32)
    out[pi] = arr.transpose(0, 2, 1, 3).reshape(2 * n_chunks, FD, 2)
    return out


def _patterns(w):
    """16 block-diagonal stationaries packed as [128, 512] bf16: pattern j in
    cols [32j, 32j+32) with w at (rows 0-63, col 2j), (rows 64-127, col
    2j+1)."""
    pat = np.zeros((P, FD), np.float32)
    for j in range(16):
        pat[:EMB, 32 * j + 2 * j] = w
        pat[EMB:, 32 * j + 2 * j + 1] = w
    return pat


def kernel(actions_idx, node_embedding, state_embedding, W_4, W_5):
    LAST_RUNS.clear()
    actions_idx = np.asarray(actions_idx)
    node_embedding = np.ascontiguousarray(np.asarray(node_embedding, dtype=np.float32))
    state_embedding = np.ascontiguousarray(np.asarray(state_embedding, dtype=np.float32))
    w4 = np.asarray(W_4, dtype=np.float32).reshape(EMB)
    w5 = np.asarray(W_5, dtype=np.float32).reshape(EMB)
    bf16 = mybir.dt.np(BF16)
    patw4 = _patterns(w4).astype(bf16)
    patw5 = _patterns(w5).astype(bf16)

    # ---- launch A: fused node+state stream (only referenced nodes staged)
    uniq, inv = np.unique(actions_idx, return_inverse=True)
    u_pc = -(-len(uniq) // NCORES)               # referenced nodes per core
    n_chunks = max(1, -(-u_pc // CHUNK_ROWS))    # 22 for the target workload
    n_slots = n_chunks * CHUNK_ROWS
    ncA = _runner(("fused", n_chunks), lambda: build_fused(n_chunks))
    inA = []
    for c in range(NCORES):
        rows = node_embedding[uniq[c * u_pc:(c + 1) * u_pc]]
        inA.append({
            "ndT": _slotT(rows, n_slots, n_chunks),
            "stT": _slotT(state_embedding[c * BATCH_PC:(c + 1) * BATCH_PC],
                          S_SLOTS, S_CHUNKS),
            "patw4": patw4,
            "patw5": patw5,
        })
    resA = _run_spmd(ncA, inA)

    tblu = np.empty((NCORES * u_pc, 2), np.float32)
    for c in range(NCORES):
        sl = slice(c * u_pc, (c + 1) * u_pc)
        tblu[sl, 0] = _unslot(resA[c]["d_out"], n_chunks)[:u_pc]
        tblu[sl, 1] = _unslot(resA[c]["a1_out"], n_chunks)[:u_pc]

    # ---- launch B: combine
    ncB = _runner("combine", build_combine)
    inB = []
    for c in range(NCORES):
        cinv = inv[c * BATCH_PC:(c + 1) * BATCH_PC]
        pairs = np.zeros((S_SLOTS, 2), np.float32)
        pairs[:BATCH_PC] = tblu[cinv]
        inB.append({
            "sv": resA[c]["s_out"],
            "t2": _slot_pairs(pairs, S_CHUNKS).astype(bf16),
        })
    resB = _run_spmd(ncB, inB)

    out = np.empty(BATCH, np.float32)
    for c in range(NCORES):
        out[c * BATCH_PC:(c + 1) * BATCH_PC] = \
            _unslot(resB[c]["q"], S_CHUNKS)[:BATCH_PC]
    return out.reshape(BATCH, 1)


# revision 37
# speedup vs baseline: 1.0921x; 1.0749x over previous
"""Trainium2 Bass kernel for nn_Decoding_43404939493634 (gnn_message_passing).

Reference computation:
    Z_a = node_embedding[actions_idx]            # [B, 64] gather
    s   = state_embedding @ W_4                  # [B, 1]
    Q   = relu(Z_a * s) @ W_5                    # [B, 1]

Algebraic restructuring: for a row with scalar s,
    relu(z * s) @ W5 = s * (relu(z) @ W5)        if s > 0
                     = s * (min(z, 0) @ W5)      if s <= 0
so with per-node values a0 = relu(node)@W5, a1 = min(node,0)@W5 (computed
on device), the per-batch-row work collapses to
    Q[b] = s[b] * ((s[b] > 0) * d[idx[b]] + a1[idx[b]]),   d = a0 - a1

All 64-wide dot products run on the Tensor engine: the host stages
state/node shards TRANSPOSED in a "slot" layout [128, cols] where
column n of chunk c holds two rows' embeddings (partitions 0-63 /
64-127), cast host-side to bf16 (halves HBM read traffic vs streaming
f32; same numerics as the previous inline-cast design).  A bf16 matmul
with a tiny block-diagonal stationary (16 patterns x 4 PE column
groups) lands each chunk's 1024 dot products on its own pair of PSUM
partitions, filling [128, 512] PSUM banks.

Two SPMD launches on 8 cores (measured 57-64 us total vs 92 us for the
previous f32-stream two-launch version; each launch carries ~6 us of
fixed framework preamble and ~3.5 us of end-barrier cost):
  launch A (fused stream, ~41-46 us): streams the REFERENCED node rows
      (~21.6k/core, 2.8 MiB/core bf16) AND the state shard (6.1
      MiB/core bf16), interleaved, via the two HWDGE queues (sync/act)
      alternating.  relu on DVE, PSUM evacuation via ScalarE Copy +
      DVE/GpSimd subs, PE warmed up with small dummy matmuls (the PE
      clock ramps 1.2 -> 2.4 GHz only after ~16 us of activity — the
      launch is PE-ramp-bound, so fusing both streams keeps PE
      saturated and the cold phase absorbs as many of the total matmul
      cycles as possible).  Offloading dot products to DVE/GpSimd
      fold-trees was tried and measured SLOWER (cross-engine dependency
      stalls); so were per-stream DMA queues and finer PSUM evacuation.
      Outputs: s table (state@W4) + per-node (d, a1) tables.
  host: t2 = tbl[actions_idx] gather (data movement only).
  launch B (combine, ~16.5 us): loads s + t2 (both bf16), computes
      q = s * ((s>0)*d + a1) on DVE in pipelined column halves.

Host work is data movement only (pad/transpose/permute/take/unique/cast);
every arithmetic op runs on device.
"""

import sys

for _p in ("/opt/trn_rl_repo",):
    if _p not in sys.path:
        sys.path.insert(0, _p)

import numpy as np

import concourse.bacc as bacc
import concourse.mybir as mybir
import concourse.tile as tile

F32 = mybir.dt.float32
BF16 = mybir.dt.bfloat16
ALU = mybir.AluOpType
COPY = mybir.ActivationFunctionType.Copy
P = 128

N_NODES = 200000
BATCH = 400000
EMB = 64
NCORES = 8

BATCH_PC = BATCH // NCORES           # 50000 rows/core

FD = 512                             # matmul moving free dim / psum bank cols
CHUNK_ROWS = 2 * FD                  # rows ("slots") covered per matmul

S_CHUNKS = -(-BATCH_PC // CHUNK_ROWS)   # 49
S_SLOTS = S_CHUNKS * CHUNK_ROWS         # 50176
S_COLS = S_CHUNKS * FD                  # 25088
S_GROUPS = -(-S_CHUNKS // 16)           # 4 psum banks for s

DMA_COLS = 4096                      # 1 MiB (bf16) per streaming DMA
HEAD_COLS = 1024


def _nc(num_devices):
    return bacc.Bacc(
        "TRN2", target_bir_lowering=False, debug=False, num_devices=num_devices
    )


def _dma_tiles(total_cols, first=0):
    """Streaming schedule: small first tiles (short latency to the first
    matmul), then 1 MiB tiles.  `first` hoists the tail columns
    [first, total_cols) to the front so the PSUM group they complete is
    evacuated early, off the combine tail."""
    sched = []
    if first:
        sched.append((first, total_cols - first))
        total_cols = first
    c0 = 0
    for w in (FD, HEAD_COLS):
        if c0 < total_cols:
            cw = min(w, total_cols - c0)
            sched.append((c0, cw))
            c0 += cw
    while c0 < total_cols:
        cw = min(DMA_COLS, total_cols - c0)
        sched.append((c0, cw))
        c0 += cw
    return sched


def build_fused(n_chunks, num_devices=NCORES):
    """Launch A: stream nodes + state (bf16), compute
    s = state@W4, a1 = node@W5 - relu(node)@W5, d = 2*relu(node)@W5 - node@W5."""
    n_cols = n_chunks * FD
    n_groups = -(-n_chunks // 16)
    nc = _nc(num_devices)
    ndT = nc.declare_dram_parameter("ndT", [P, n_cols], BF16, isOutput=False)
    stT = nc.declare_dram_parameter("stT", [P, S_COLS], BF16, isOutput=False)
    patw4 = nc.declare_dram_parameter("patw4", [P, FD], BF16, isOutput=False)
    patw5 = nc.declare_dram_parameter("patw5", [P, FD], BF16, isOutput=False)
    d_out = nc.declare_dram_parameter("d_out", [P, FD], F32, isOutput=True)
    a1_out = nc.declare_dram_parameter("a1_out", [P, FD], F32, isOutput=True)
    s_out = nc.declare_dram_parameter("s_out", [P, FD], BF16, isOutput=True)

    with tile.TileContext(nc) as tc:
        with (
            tc.tile_pool(name="const", bufs=1) as cpool,
            tc.tile_pool(name="nwork", bufs=5) as npool,
            tc.tile_pool(name="swork", bufs=9) as spool,
            tc.tile_pool(name="psum", bufs=1, space="PSUM") as ppool,
        ):
            p4 = cpool.tile([P, FD], BF16, tag="p4")
            nc.sync.dma_start(out=p4[:], in_=patw4[:])
            p5 = cpool.tile([P, FD], BF16, tag="p5")
            nc.scalar.dma_start(out=p5[:], in_=patw5[:])

            ps_s = [ppool.tile([P, FD], F32, tag=f"ps_s{g}", name=f"ps_s{g}")
                    for g in range(S_GROUPS)]
            ps_a0 = [ppool.tile([P, FD], F32, tag=f"ps_a0{g}", name=f"ps_a0{g}")
                     for g in range(n_groups)]
            ps_s5 = [ppool.tile([P, FD], F32, tag=f"ps_s5{g}", name=f"ps_s5{g}")
                     for g in range(n_groups)]

            # PE warmup: dummy matmuls during the head window so the HAM
            # clock-gate releases (1.2 -> 2.4 GHz) earlier; measured ~3-4us
            # faster than without.  Uses partitions 0-31 of g2's bank (real
            # g2 data lands in partitions 64-95, and g2 starts last).
            warm = cpool.tile([P, FD], BF16, tag="warm")
            nc.vector.memset(warm[:], 0.0)
            for _ in range(7):
                nc.tensor.matmul(
                    ps_s[2][0:32, 0:256], warm[:, 0:32], warm[:, 0:256],
                    start=True, stop=True, skip_group_check=True,
                    tile_position=(0, 0),
                )

            # evacuation staging tiles
            svt = cpool.tile([P, FD], BF16, tag="svt")
            a0t = cpool.tile([P, FD], F32, tag="a0t")
            a1t = cpool.tile([P, FD], F32, tag="a1t")
            dt_ = cpool.tile([P, FD], F32, tag="dt")

            # unified stream schedule: state head (incl. hoisted lone g3
            # chunk) first, then alternate node/state tiles (measured best;
            # all-nodes-first and all-state-first are both ~2-3us slower)
            nsched = [("n",) + t for t in _dma_tiles(n_cols)]
            ssched = [("s",) + t
                      for t in _dma_tiles(S_COLS, first=(S_CHUNKS - 1) * FD)]
            sched = []
            while ssched or nsched:
                if ssched:
                    sched.append(ssched.pop(0))
                if nsched:
                    sched.append(nsched.pop(0))

            s_left = [16, 16, 16, S_CHUNKS - 48]  # chunks remaining per group
            n_left = [min(16, n_chunks - 16 * g) for g in range(n_groups)]

            qi = 0
            for kind, c0, cw in sched:
                eng = nc.sync if (qi % 2 == 0) else nc.scalar
                qi += 1
                src = ndT if kind == "n" else stT
                pool = npool if kind == "n" else spool
                tb = pool.tile([P, cw], BF16, tag=f"{kind}b")
                eng.dma_start(out=tb[:], in_=src[:, c0:c0 + cw])
                if kind == "n":
                    rl = npool.tile([P, cw], BF16, tag="rl")
                    nc.vector.tensor_scalar_max(out=rl[:], in0=tb[:], scalar1=0.0)
                base = c0 // FD
                done = []
                for k in range(cw // FD):
                    c = base + k
                    g, j = divmod(c, 16)
                    if kind == "s":
                        nc.tensor.matmul(
                            ps_s[g][32 * g:32 * g + 32, :],
                            p4[:, 32 * j:32 * j + 32],
                            tb[:, k * FD:(k + 1) * FD],
                            start=(j == 0),
                            stop=(j == 15) or (c == S_CHUNKS - 1),
                            skip_group_check=True,
                            tile_position=(0, 32 * g),
                        )
                        s_left[g] -= 1
                        if s_left[g] == 0:
                            done.append(("s", g))
                    else:
                        st_flags = dict(
                            start=(j == 0),
                            stop=(j == 15) or (c == n_chunks - 1),
                            skip_group_check=True,
                            tile_position=(0, 32 * g),
                        )
                        # s5 first: it reads the raw tile, no relu dep
                        nc.tensor.matmul(
                            ps_s5[g][32 * g:32 * g + 32, :],
                            p5[:, 32 * j:32 * j + 32],
                            tb[:, k * FD:(k + 1) * FD],
                            **st_flags,
                        )
                        nc.tensor.matmul(
                            ps_a0[g][32 * g:32 * g + 32, :],
                            p5[:, 32 * j:32 * j + 32],
                            rl[:, k * FD:(k + 1) * FD],
                            **st_flags,
                        )
                        n_left[g] -= 1
                        if n_left[g] == 0:
                            done.append(("n", g))
                # evacuate completed PSUM groups (PSUM reads on scalar/DVE —
                # gpsimd cannot touch PSUM; SBUF-only subtract on gpsimd)
                for dk, g in done:
                    sl = slice(32 * g, 32 * g + 32)
                    if dk == "s":
                        nc.scalar.activation(
                            out=svt[sl, :], in_=ps_s[g][sl, :], func=COPY
                        )
                        nc.sync.dma_start(out=s_out[sl, :], in_=svt[sl, :])
                    else:
                        nc.scalar.activation(
                            out=a0t[sl, :], in_=ps_a0[g][sl, :], func=COPY
                        )
                        nc.vector.tensor_tensor(
                            out=a1t[sl, :], in0=ps_s5[g][sl, :], in1=a0t[sl, :],
                            op=ALU.subtract,
                        )
                        nc.gpsimd.tensor_tensor(
                            out=dt_[sl, :], in0=a0t[sl, :], in1=a1t[sl, :],
                            op=ALU.subtract,
                        )
                        nc.scalar.dma_start(out=a1_out[sl, :], in_=a1t[sl, :])
                        nc.scalar.dma_start(out=d_out[sl, :], in_=dt_[sl, :])
    nc.compile()
    return nc


def build_combine(num_devices=NCORES):
    """Launch B: q = s * ((s>0)*d + a1), with (d, a1) host-gathered per row."""
    nc = _nc(num_devices)
    sv = nc.declare_dram_parameter("sv", [P, FD], BF16, isOutput=False)
    t2 = nc.declare_dram_parameter("t2", [P, FD, 2], BF16, isOutput=False)
    q = nc.declare_dram_parameter("q", [P, FD], F32, isOutput=True)

    NH = 2                                   # column halves, pipelined
    HW_ = FD // NH
    with tile.TileContext(nc) as tc:
        with tc.tile_pool(name="const", bufs=1) as cpool:
            svt = cpool.tile([P, FD], BF16, tag="svt")
            t2t = cpool.tile([P, FD, 2], BF16, tag="t2t")
            posm = cpool.tile([P, FD], F32, tag="posm")
            sel = cpool.tile([P, FD], F32, tag="sel")
            qt = cpool.tile([P, FD], F32, tag="qt")
            for h in range(NH):
                cs = slice(h * HW_, (h + 1) * HW_)
                nc.sync.dma_start(out=t2t[:, cs, :], in_=t2[:, cs, :])
                nc.scalar.dma_start(out=svt[:, cs], in_=sv[:, cs])
            for h in range(NH):
                cs = slice(h * HW_, (h + 1) * HW_)
                nc.vector.scalar_tensor_tensor(
                    out=posm[:, cs], in0=svt[:, cs], scalar=0.0,
                    in1=t2t[:, cs, 0], op0=ALU.is_gt, op1=ALU.mult,
                )
                nc.vector.tensor_tensor(
                    out=sel[:, cs], in0=posm[:, cs], in1=t2t[:, cs, 1],
                    op=ALU.add,
                )
                nc.vector.tensor_tensor(
                    out=qt[:, cs], in0=svt[:, cs], in1=sel[:, cs], op=ALU.mult
                )
                nc.sync.dma_start(out=q[:, cs], in_=qt[:, cs])
    nc.compile()
    return nc


# ---------------------------------------------------------------------------
# host-side staging (data movement only) + execution

_CACHE = {}
LAST_RUNS = []  # BassKernelResults of each launch in the last kernel() call


def _runner(key, build_fn):
    if key not in _CACHE:
        _CACHE[key] = build_fn()
    return _CACHE[key]


def _run_spmd(nc, in_maps):
    from concourse.bass_utils import run_bass_kernel_spmd

    r = run_bass_kernel_spmd(nc, in_maps, core_ids=list(range(NCORES)))
    LAST_RUNS.append(r)
    return r.results


def _slotT(rows, n_slots, nch):
    """[n, 64] -> transposed slot layout [128, n_slots//2] bf16: column of
    chunk c, col n holds rows (1024c+2n) on partitions 0-63 and (1024c+2n+1)
    on 64-127."""
    n = rows.shape[0]
    buf = np.zeros((n_slots, EMB), np.float32)
    buf[:n] = rows
    arr = buf.reshape(nch, FD, 2, EMB)           # [c, n, h, e]
    return np.ascontiguousarray(
        arr.transpose(2, 3, 0, 1).reshape(P, nch * FD)
    ).astype(mybir.dt.np(BF16))


def _pidx(n_chunks):
    """Partition index of (chunk c, half h) in the psum/slot output layout."""
    c = np.arange(n_chunks)[:, None]
    h = np.arange(2)[None, :]
    return (32 * (c // 16) + 2 * (c % 16) + h)   # [n_chunks, 2]


def _unslot(mat, n_chunks):
    """[128, 512] device output -> flat [n_chunks*1024] slot-ordered values."""
    pi = _pidx(n_chunks).reshape(-1)             # [2*n_chunks]
    v = mat[pi, :].reshape(n_chunks, 2, FD)      # [c, h, n]
    return np.ascontiguousarray(v.transpose(0, 2, 1)).reshape(-1)


def _slot_pairs(pairs, n_chunks):
    """[n_slots, 2] per-slot values -> [128, 512, 2] device layout."""
    pi = _pidx(n_chunks).reshape(-1)
    arr = pairs.reshape(n_chunks, FD, 2, 2)      # [c, n, h, v]
    out = np.zeros((P, FD, 2), np.float32)
    out[pi] = arr.transpose(0, 2, 1, 3).reshape(2 * n_chunks, FD, 2)
    return out


def _patterns(w):
    """16 block-diagonal stationaries packed as [128, 512] bf16: pattern j in
    cols [32j, 32j+32) with w at (rows 0-63, col 2j), (rows 64-127, col
    2j+1)."""
    pat = np.zeros((P, FD), np.float32)
    for j in range(16):
        pat[:EMB, 32 * j + 2 * j] = w
        pat[EMB:, 32 * j + 2 * j + 1] = w
    return pat


def kernel(actions_idx, node_embedding, state_embedding, W_4, W_5):
    LAST_RUNS.clear()
    actions_idx = np.asarray(actions_idx)
    node_embedding = np.ascontiguousarray(np.asarray(node_embedding, dtype=np.float32))
    state_embedding = np.ascontiguousarray(np.asarray(state_embedding, dtype=np.float32))
    w4 = np.asarray(W_4, dtype=np.float32).reshape(EMB)
    w5 = np.asarray(W_5, dtype=np.float32).reshape(EMB)
    bf16 = mybir.dt.np(BF16)
    patw4 = _patterns(w4).astype(bf16)
    patw5 = _patterns(w5).astype(bf16)

    # ---- launch A: fused node+state stream (only referenced nodes staged)
    uniq, inv = np.unique(actions_idx, return_inverse=True)
    u_pc = -(-len(uniq) // NCORES)               # referenced nodes per core
    n_chunks = max(1, -(-u_pc // CHUNK_ROWS))    # 22 for the target workload
    n_slots = n_chunks * CHUNK_ROWS
    ncA = _runner(("fused", n_chunks), lambda: build_fused(n_chunks))
    inA = []
    for c in range(NCORES):
        rows = node_embedding[uniq[c * u_pc:(c + 1) * u_pc]]
        inA.append({
            "ndT": _slotT(rows, n_slots, n_chunks),
            "stT": _slotT(state_embedding[c * BATCH_PC:(c + 1) * BATCH_PC],
                          S_SLOTS, S_CHUNKS),
            "patw4": patw4,
            "patw5": patw5,
        })
    resA = _run_spmd(ncA, inA)

    tblu = np.empty((NCORES * u_pc, 2), np.float32)
    for c in range(NCORES):
        sl = slice(c * u_pc, (c + 1) * u_pc)
        tblu[sl, 0] = _unslot(resA[c]["d_out"], n_chunks)[:u_pc]
        tblu[sl, 1] = _unslot(resA[c]["a1_out"], n_chunks)[:u_pc]

    # ---- launch B: combine
    ncB = _runner("combine", build_combine)
    inB = []
    for c in range(NCORES):
        cinv = inv[c * BATCH_PC:(c + 1) * BATCH_PC]
        pairs = np.zeros((S_SLOTS, 2), np.float32)
        pairs[:BATCH_PC] = tblu[cinv]
        inB.append({
            "sv": resA[c]["s_out"],
            "t2": _slot_pairs(pairs, S_CHUNKS).astype(bf16),
        })
    resB = _run_spmd(ncB, inB)

    out = np.empty(BATCH, np.float32)
    for c in range(NCORES):
        out[c * BATCH_PC:(c + 1) * BATCH_PC] = \
            _unslot(resB[c]["q"], S_CHUNKS)[:BATCH_PC]
    return out.reshape(BATCH, 1)
